# revision 12
# baseline (speedup 1.0000x reference)
"""DMSAD loss kernel for Trainium2 (8 NeuronCores, data-parallel over batch).

Computes mean over B rows of:
    dist_i = max(min_j ||x_i - c_j||^2, 0)
    loss_i = dist_i                 if st_i == 0
             dist_i + EPS           if st_i == 1
             1 / (dist_i + EPS)     if st_i == -1

Per core (B_SH = 16384 rows, D = 256, C = 128), engine pipeline:
  - DMA (SWDGE/gpsimd): casting fp32->bf16 HBM loads -- the 16.8 MB fp32
    read paces at the ~47us HBM roofline while landing bf16 directly in
    SBUF, deleting the ACT/DVE cast stage entirely.
  - PE: transposes are issued as NORMAL matmuls against an identity rhs
    (out = lhsT.T @ I) rather than transpose-mode ops -- transpose-mode
    does not count as PE-busy for the HAM clock gate, and with it the
    PE kept falling back to the cold 1.2 GHz clock mid-stream.  Normal
    matmuls keep HAM at 8/8 (2.4 GHz).  Output lands fp32 in PSUM
    (2-tile minigroups, 1 bank each); main bf16 matmuls G += -2 x.c^T;
    a K=2 ones x [c2_hi; c2_lo] matmul folds the center norms in; for
    most groups a rank-1 ones-matmul of the squared transposed tiles
    folds x2 into G so dist falls out of the min directly.
  - ACT: casting fp32->bf16 PSUM->SBUF copies of the transposed tiles,
    Square for part of the sqt work and the sqred groups.
  - DVE: part of the copies/sqt, batched min-reduce per 8-tile G tile,
    half-fold + reduce-add x2 for sqred groups, endgame in small chunks
    overlapped with the main loop.
A final ones-matmul collapses the per-partition loss sums to one scalar
per core (single-descriptor 4-byte out DMA); host adds the 8 partials.
"""

from contextlib import ExitStack, nullcontext

import numpy as np

import concourse.bass as bass
import concourse.tile as tile
from concourse import bacc, mybir
from concourse.bass_utils import run_bass_kernel_spmd
from concourse.masks import make_identity

N_CORES = 8
B = 131072
D = 256
C = 128
P = 128
B_SH = B // N_CORES          # 16384 rows per core
NT = B_SH // P               # 128 b-tiles of 128 rows
MINI = 2                     # b-tiles per transpose PSUM minigroup (1 bank fp32)
PSUM_GROUP = 4               # b-tiles per G PSUM bank
DMA_GROUP = 8                # b-tiles per input DMA (1 MiB fp32 reads)
G_TILES = 8                  # b-tiles per G PSUM tile (2 banks, one min-reduce)
ETA = 1.0
EPS = 1e-6

# ---- engine-balance knobs -------------------------------------------------
# x2-on-PE pattern: group pi uses the PE rank-1 path unless pi % 3 == X2_SKIP
X2_SKIP = 2
# sqt (squares of transposed tiles) engine per group: ACT if pi % 3 == 0
SQT_ACT_MOD = 0
# PSUM->SBUF casting copies: minigroup indices (0..3) that go to DVE
COPY_DVE_MINIS = (3,)
# endgame trigger points (t0+ntile values) and column chunks
EG_PLAN = [(32, 0, 32), (64, 32, 64), (96, 64, 96), (120, 96, 120)]
EG_FINAL = (120, 128)

F32 = mybir.dt.float32
BF16 = mybir.dt.bfloat16
AF = mybir.ActivationFunctionType
ALU = mybir.AluOpType

_cached_nc = {}


def _emit(ctx: ExitStack, tc, x_d, c_d, st_d, out_d, repeat: int = 1,
          hw_loop: int = 1):
    nc = tc.nc

    const = ctx.enter_context(tc.tile_pool(name="const", bufs=1))
    xbpool = ctx.enter_context(tc.tile_pool(name="xb", bufs=6))
    sqpool = ctx.enter_context(tc.tile_pool(name="sq", bufs=3))
    xtps = ctx.enter_context(tc.tile_pool(name="xtps", bufs=3, space="PSUM"))
    xtsb = ctx.enter_context(tc.tile_pool(name="xtsb", bufs=3))
    # G lives in 2-bank [P, 8, C] tiles so the min-reduce batches a whole
    # DMA group.
    gps = ctx.enter_context(tc.tile_pool(name="gps", bufs=2, space="PSUM"))
    scr_ps = ctx.enter_context(tc.tile_pool(name="scrps", bufs=1, space="PSUM"))
    endp = ctx.enter_context(tc.tile_pool(name="endp", bufs=1))

    # ---- one-time prep -------------------------------------------------
    # x loads are casting fp32->bf16 SWDGE DMAs (gpsimd): queue the first
    # groups immediately -- they need no prep and pace the whole kernel.
    pre_x8 = []
    for gd in range(2):
        src0 = x_d[gd * 4 * P:(gd + 1) * 4 * P, :]
        src0 = src0.rearrange("(p t) d -> p t d", t=4)
        x80 = xbpool.tile([P, DMA_GROUP, D], BF16, tag="xb")
        nc.gpsimd.dma_start(x80[:, :4, :], src0)
        pre_x8.append(x80)

    ident_bf = const.tile([P, P], BF16)
    make_identity(nc, ident_bf[:])
    ident_f32 = const.tile([P, P], F32)
    make_identity(nc, ident_f32[:])

    # warm the ACT Square table set while DMAs are in flight (the
    # ACT_TABLE_LOAD costs ~2.7us and would otherwise sit in the prep
    # critical path at first use)
    warm = const.tile([1, 1], F32)
    nc.scalar.activation(warm[:], ident_f32[0:1, 0:1], AF.Square)

    c_sb = const.tile([C, D], F32)
    nc.sync.dma_start(c_sb[:], c_d[:])

    # c2 = rowsum(c^2) as a [128, 1] fp32 column
    c_sq = const.tile([C, D], F32)
    c2col = const.tile([C, 1], F32)
    nc.scalar.activation(c_sq[:], c_sb[:], AF.Square, accum_out=c2col[:])

    # (-2c) in bf16, then its transpose cT [d-chunk partitions, k, centers]
    cm2 = const.tile([C, D], BF16)
    nc.vector.tensor_scalar_mul(cm2[:], c_sb[:], -2.0)
    ct_ps = scr_ps.tile([P, 2, C], BF16, tag="scratch")
    for k in range(2):
        nc.tensor.transpose(ct_ps[:, k, :], cm2[:, k * P:(k + 1) * P], ident_bf[:])
    cT = const.tile([P, 2, C], BF16)
    nc.vector.tensor_copy(cT[:], ct_ps[:])

    # c2 as two bf16 K-rows (hi + lo) so a K=2 ones-matmul adds fp32-accurate c2
    c2t_ps = scr_ps.tile([1, C], F32, tag="scratch")
    nc.tensor.transpose(c2t_ps[:], c2col[:], ident_f32[:])
    c2row_f = const.tile([1, C], F32)
    nc.vector.tensor_copy(c2row_f[:], c2t_ps[:])
    c2rows = const.tile([2, C], BF16)
    nc.vector.tensor_copy(c2rows[0:1, :], c2row_f[:])
    c2hi_f = const.tile([1, C], F32)
    nc.vector.tensor_copy(c2hi_f[:], c2rows[0:1, :])
    c2lo_f = const.tile([1, C], F32)
    nc.vector.tensor_tensor(c2lo_f[:], c2row_f[:], c2hi_f[:], op=ALU.subtract)
    # engines can't write at base partition 1; a casting SBUF->SBUF DMA can
    nc.gpsimd.dma_start(c2rows[1:2, :], c2lo_f[:])

    ones2 = const.tile([2, C], BF16)
    nc.vector.memset(ones2[:], 1.0)
    ones_col = const.tile([P, 1], F32)
    nc.vector.memset(ones_col[:], 1.0)

    # c2rows replicated PSUM_GROUP times for the single N=512 c2 matmul
    c2rows4 = const.tile([2, PSUM_GROUP, C], BF16)
    for i in range(PSUM_GROUP):
        nc.vector.tensor_copy(c2rows4[:, i, :], c2rows[:])

    # all-ones [d, c] rhs for the PE-side x2 rank-1 accumulation
    ones_dc = const.tile([P, C], BF16)
    nc.vector.memset(ones_dc[:], 1.0)

    # semi_target: the HOST pre-permutes st into the x row mapping
    # (st_pre[p*NT + col] = st[row(p, col)], see make_in_maps), so one
    # contiguous 512B-per-partition DMA loads it.  The old direct load
    # of the scattered layout was 2048 32-byte descriptors (~14us) and
    # stalled the DVE queue behind the endgame's st-dependent ops.
    st_sb = const.tile([P, NT], F32)
    nc.sync.dma_start(st_sb[:], st_d[:].rearrange("(p j) -> p j", p=P))

    # per-b-tile accumulators: column j <-> b-tile j, partition p <-> row in tile
    mw = const.tile([P, NT], F32)
    x2w = const.tile([P, NT], F32)
    n_eg = len(EG_PLAN) + 1
    lsum2 = const.tile([P, n_eg], F32)
    # PE-x2 groups fold x2 into G before the min; their x2w columns
    # must read as zero in the endgame's dist = x2w + mw
    nc.vector.memset(x2w[:], 0.0)

    # ---- endgame (runs in chunks; all but the last overlap the main loop)
    def endgame_chunk(h, lo, hi):
        cols = slice(lo, hi)
        W = hi - lo
        dist = endp.tile([P, W], F32, tag=f"dist{h}")
        nc.vector.tensor_tensor(dist[:], x2w[:, cols], mw[:, cols], op=ALU.add)
        nc.vector.tensor_scalar_max(dist[:], dist[:], 0.0)
        dp = endp.tile([P, W], F32, tag=f"dp{h}")
        nc.vector.tensor_scalar_add(dp[:], dist[:], EPS)
        r = endp.tile([P, W], F32, tag=f"r{h}")
        nc.vector.reciprocal(r[:], dp[:])
        # loss = dist + min(st,0)*(dist - r) + max(st,0)*EPS
        t1 = endp.tile([P, W], F32, tag=f"t1{h}")
        nc.vector.tensor_tensor(t1[:], dist[:], r[:], op=ALU.subtract)
        mneg = endp.tile([P, W], F32, tag=f"mneg{h}")
        nc.vector.tensor_scalar_min(mneg[:], st_sb[:, cols], 0.0)
        t2 = endp.tile([P, W], F32, tag=f"t2{h}")
        nc.vector.tensor_tensor(t2[:], mneg[:], t1[:], op=ALU.mult)
        t3 = endp.tile([P, W], F32, tag=f"t3{h}")
        nc.vector.tensor_tensor(t3[:], dist[:], t2[:], op=ALU.add)
        epsq = endp.tile([P, W], F32, tag=f"eq{h}")
        nc.vector.tensor_scalar(epsq[:], st_sb[:, cols], 0.0, EPS, op0=ALU.max,
                                op1=ALU.mult)
        losses = endp.tile([P, W], F32, tag=f"lo{h}")
        nc.vector.tensor_tensor(losses[:], t3[:], epsq[:], op=ALU.add)
        nc.vector.tensor_reduce(lsum2[:, h:h + 1], losses[:],
                                axis=mybir.AxisListType.X, op=ALU.add)

    # ---- main loop -----------------------------------------------------
    # Edge groups are 4 tiles (0.5 MiB loads) so the pipeline ramps in and
    # drains out faster; the steady-state middle uses 8-tile groups.
    if repeat == 1 and hw_loop == 1:
        plan = [(t, 4) for t in range(0, 16, 4)]
        plan += [(t, 8) for t in range(16, 112, 8)]
        plan += [(t, 4) for t in range(112, 128, 4)]
    else:
        plan = [(t, 8) for t in range(0, NT, 8)]

    mini_idx = 0
    with tc.For_i(0, hw_loop, 1) if hw_loop > 1 else nullcontext():
     for _rep in range(repeat):
      for pi, (t0, ntile) in enumerate(plan):
        src = x_d[t0 * P:(t0 + ntile) * P, :]
        # x8[:, t, :] = b-tile t0+t in natural row order (partition p =
        # row (t0+t)*128 + p); 1 KiB descriptors per (partition, tile)
        src = src.rearrange("(t p) d -> p t d", t=ntile)
        if repeat == 1 and hw_loop == 1 and pi < len(pre_x8):
            x8 = pre_x8[pi]
        else:
            x8 = xbpool.tile([P, DMA_GROUP, D], BF16, tag="xb")
            nc.gpsimd.dma_start(x8[:, :ntile, :], src)

        cols = slice(t0, t0 + ntile)
        x2_on_pe = (pi % 3 != X2_SKIP)
        if not x2_on_pe:
            # sqred path: Square(x8) on ACT, then half-fold (bf16 2x TT)
            # + strided reduce-add on DVE
            sq = sqpool.tile([P, DMA_GROUP, D], BF16, tag="sq")
            nc.scalar.activation(sq[:, :ntile, :], x8[:, :ntile, :], AF.Square)
            s1 = sqpool.tile([P, DMA_GROUP, P], BF16, tag="s1")
            nc.vector.tensor_tensor(
                s1[:, :ntile, :], sq[:, :ntile, 0:P], sq[:, :ntile, P:D],
                op=ALU.add,
            )
            nc.vector.tensor_reduce(
                x2w[:, cols], s1[:, :ntile, :], axis=mybir.AxisListType.X,
                op=ALU.add,
            )

        g_ps = gps.tile([P, G_TILES, C], F32)
        for gp in range(ntile // PSUM_GROUP):
            tiles = [gp * PSUM_GROUP + t for t in range(PSUM_GROUP)]

            xt_ps = xtps.tile([P, PSUM_GROUP, 2, P], BF16)
            for i, t in enumerate(tiles):
                for k in range(2):
                    nc.tensor.transpose(
                        xt_ps[:, i, k, :], x8[:, t, k * P:(k + 1) * P],
                        ident_bf[:],
                    )
            xt_t = xtsb.tile([P, PSUM_GROUP, 2, P], BF16)
            # bf16 stays bf16 in PSUM; move it as fp32 pairs (half the
            # elements; exact on normals)
            cp_src = xt_ps[:].bitcast(F32)
            cp_dst = xt_t[:].bitcast(F32)
            if (mini_idx % 4) in COPY_DVE_MINIS:
                nc.vector.tensor_copy(cp_dst, cp_src)
            else:
                nc.scalar.copy(cp_dst, cp_src)
            mini_idx += 1

            if x2_on_pe:
                sqt = sqpool.tile([P, PSUM_GROUP, 2, P], BF16, tag="sqt")
                if pi % 3 == SQT_ACT_MOD:
                    nc.scalar.activation(sqt[:], xt_t[:], AF.Square)
                else:
                    nc.vector.tensor_tensor(sqt[:], xt_t[:], xt_t[:],
                                            op=ALU.mult)

            g_half = g_ps[:, gp * PSUM_GROUP:(gp + 1) * PSUM_GROUP, :]
            nc.tensor.matmul(
                g_half.rearrange("p t c -> p (t c)"),
                lhsT=ones2[:], rhs=c2rows4[:].rearrange("p t c -> p (t c)"),
                start=True, stop=False,
            )
            for i in range(PSUM_GROUP):
                last_tile = i == PSUM_GROUP - 1
                nc.tensor.matmul(
                    g_half[:, i, :], lhsT=xt_t[:, i, 0, :], rhs=cT[:, 0, :],
                    start=False, stop=False,
                )
                nc.tensor.matmul(
                    g_half[:, i, :], lhsT=xt_t[:, i, 1, :], rhs=cT[:, 1, :],
                    start=False, stop=(last_tile and not x2_on_pe),
                )
                if x2_on_pe:
                    nc.tensor.matmul(
                        g_half[:, i, :], lhsT=sqt[:, i, 0, :], rhs=ones_dc[:],
                        start=False, stop=False,
                    )
                    nc.tensor.matmul(
                        g_half[:, i, :], lhsT=sqt[:, i, 1, :], rhs=ones_dc[:],
                        start=False, stop=last_tile,
                    )

        nc.vector.tensor_reduce(
            mw[:, cols], g_ps[:, :ntile, :], axis=mybir.AxisListType.X,
            op=ALU.min,
        )

        if repeat == 1 and hw_loop == 1:
            for h, (trig, lo, hi) in enumerate(EG_PLAN):
                if t0 + ntile == trig:
                    endgame_chunk(h, lo, hi)

    endgame_chunk(len(EG_PLAN), *EG_FINAL)
    lacc = lsum2[:, 0:1]
    lsum_t = None
    for h in range(1, n_eg):
        nxt = endp.tile([P, 1], F32, tag=f"ls{h}")
        nc.vector.tensor_tensor(nxt[:], lacc, lsum2[:, h:h + 1], op=ALU.add)
        lacc = nxt[:]
        lsum_t = nxt
    # single-descriptor 4-byte out DMA: a [128,1] out would be 128 tiny
    # descriptors whose completion receipt stalls the end barrier ~7us
    total_ps = scr_ps.tile([1, 1], F32, tag="scratch")
    nc.tensor.matmul(total_ps[:], lhsT=ones_col[:], rhs=lsum_t[:])
    total_sb = endp.tile([1, 1], F32)
    nc.vector.tensor_copy(total_sb[:], total_ps[:])
    nc.sync.dma_start(out_d[:], total_sb[:])


def build_nc(repeat: int = 1, hw_loop: int = 1, internal_x: bool = False):
    key = (repeat, hw_loop, internal_x)
    if key in _cached_nc:
        return _cached_nc[key]
    nc = bacc.Bacc(
        "TRN2",
        target_bir_lowering=False,
        debug=False,
        enable_asserts=False,
        num_devices=N_CORES,
    )
    if internal_x:
        # timing-only builds: x is internal (uninitialized) DRAM so bench
        # calls don't upload 128 MiB; compute timing is data-independent
        x_d = nc.dram_tensor("x", [B_SH, D], F32).ap()
    else:
        x_d = nc.dram_tensor("x", [B_SH, D], F32, kind="ExternalInput").ap()
    c_d = nc.dram_tensor("c", [C, D], F32, kind="ExternalInput").ap()
    st_d = nc.dram_tensor("st", [B_SH], F32, kind="ExternalInput").ap()
    out_d = nc.dram_tensor("out", [1, 1], F32, kind="ExternalOutput").ap()

    with tile.TileContext(nc) as tc:
        with ExitStack() as ctx:
            _emit(ctx, tc, x_d, c_d, st_d, out_d, repeat=repeat, hw_loop=hw_loop)
    nc.compile()
    _cached_nc[key] = nc
    return nc


_ST_IDX = None


def _st_index():
    # row index feeding st_sb[p, col]: in an ntile-tile group at tile t0,
    # batch row t0*128 + p*ntile + t sits at column t0 + t
    global _ST_IDX
    if _ST_IDX is None:
        idx = np.empty((P, NT), dtype=np.int64)
        p = np.arange(P)[:, None]
        for lo, hi, tt in ((0, 16, 4), (16, 112, 8), (112, 128, 4)):
            for g0 in range(lo, hi, tt):
                t = np.arange(tt)[None, :]
                idx[:, g0:g0 + tt] = g0 * P + p * tt + t
        _ST_IDX = idx.ravel()
    return _ST_IDX


def make_in_maps(x, c, stf):
    idx = _st_index()
    return [
        {
            "x": np.ascontiguousarray(x[i * B_SH:(i + 1) * B_SH]),
            "c": c,
            "st": np.ascontiguousarray(stf[i * B_SH:(i + 1) * B_SH][idx]),
        }
        for i in range(N_CORES)
    ]


def kernel(**inputs) -> np.ndarray:
    x = np.ascontiguousarray(np.asarray(inputs["input"], dtype=np.float32))
    c = np.ascontiguousarray(np.asarray(inputs["c"], dtype=np.float32))
    stf = np.asarray(inputs["semi_target"]).astype(np.float32)

    nc = build_nc()
    res = run_bass_kernel_spmd(nc, make_in_maps(x, c, stf), list(range(N_CORES)))
    total = sum(float(r["out"][0, 0]) for r in res.results)
    return np.asarray(np.float32(total / B))


# revision 13
# speedup vs baseline: 1.0422x; 1.0422x over previous
"""DMSAD loss kernel for Trainium2 (8 NeuronCores, data-parallel over batch).

Computes mean over B rows of:
    dist_i = max(min_j ||x_i - c_j||^2, 0)
    loss_i = dist_i                 if st_i == 0
             dist_i + EPS           if st_i == 1
             1 / (dist_i + EPS)     if st_i == -1

Per core (B_SH = 16384 rows, D = 256, C = 128), engine pipeline:
  - DMA (SWDGE/gpsimd): casting fp32->bf16 HBM loads -- the 16.8 MB fp32
    read paces at the ~47us HBM roofline while landing bf16 directly in
    SBUF, deleting the ACT/DVE cast stage entirely.
  - PE: transposes are issued as NORMAL matmuls against an identity rhs
    (out = lhsT.T @ I) rather than transpose-mode ops -- transpose-mode
    does not count as PE-busy for the HAM clock gate, and with it the
    PE kept falling back to the cold 1.2 GHz clock mid-stream.  Normal
    matmuls keep HAM at 8/8 (2.4 GHz).  Output lands fp32 in PSUM
    (2-tile minigroups, 1 bank each); main bf16 matmuls G += -2 x.c^T;
    a K=2 ones x [c2_hi; c2_lo] matmul folds the center norms in; for
    most groups a rank-1 ones-matmul of the squared transposed tiles
    folds x2 into G so dist falls out of the min directly.
  - ACT: casting fp32->bf16 PSUM->SBUF copies of the transposed tiles,
    Square for part of the sqt work and the sqred groups.
  - DVE: part of the copies/sqt, batched min-reduce per 8-tile G tile,
    half-fold + reduce-add x2 for sqred groups, endgame in small chunks
    overlapped with the main loop.
A final ones-matmul collapses the per-partition loss sums to one scalar
per core (single-descriptor 4-byte out DMA); host adds the 8 partials.
"""

from contextlib import ExitStack, nullcontext

import numpy as np

import concourse.bass as bass
import concourse.tile as tile
from concourse import bacc, mybir
from concourse.bass_utils import run_bass_kernel_spmd
from concourse.masks import make_identity

N_CORES = 8
B = 131072
D = 256
C = 128
P = 128
B_SH = B // N_CORES          # 16384 rows per core
NT = B_SH // P               # 128 b-tiles of 128 rows
MINI = 2                     # b-tiles per transpose PSUM minigroup (1 bank fp32)
PSUM_GROUP = 4               # b-tiles per G PSUM bank
DMA_GROUP = 8                # b-tiles per input DMA (1 MiB fp32 reads)
G_TILES = 8                  # b-tiles per G PSUM tile (2 banks, one min-reduce)
ETA = 1.0
EPS = 1e-6

# ---- engine-balance knobs -------------------------------------------------
# x2-on-PE pattern: group pi uses the PE rank-1 path unless pi % 3 == X2_SKIP
X2_SKIP = 2
# sqt (squares of transposed tiles) engine per group: ACT if pi % 3 == 0
SQT_ACT_MOD = 0
# PSUM->SBUF casting copies: minigroup indices (0..3) that go to DVE
COPY_DVE_MINIS = (3,)
# endgame trigger points (t0+ntile values) and column chunks
EG_PLAN = [(32, 0, 32), (64, 32, 64), (96, 64, 96), (120, 96, 120)]
EG_FINAL = (120, 128)

F32 = mybir.dt.float32
BF16 = mybir.dt.bfloat16
AF = mybir.ActivationFunctionType
ALU = mybir.AluOpType

_cached_nc = {}


def _emit(ctx: ExitStack, tc, x_d, c_d, st_d, out_d, repeat: int = 1,
          hw_loop: int = 1):
    nc = tc.nc

    const = ctx.enter_context(tc.tile_pool(name="const", bufs=1))
    xbpool = ctx.enter_context(tc.tile_pool(name="xb", bufs=6))
    sqpool = ctx.enter_context(tc.tile_pool(name="sq", bufs=3))
    xtps = ctx.enter_context(tc.tile_pool(name="xtps", bufs=3, space="PSUM"))
    xtsb = ctx.enter_context(tc.tile_pool(name="xtsb", bufs=3))
    # G lives in 2-bank [P, 8, C] tiles so the min-reduce batches a whole
    # DMA group.
    gps = ctx.enter_context(tc.tile_pool(name="gps", bufs=2, space="PSUM"))
    scr_ps = ctx.enter_context(tc.tile_pool(name="scrps", bufs=1, space="PSUM"))
    endp = ctx.enter_context(tc.tile_pool(name="endp", bufs=1))

    # ---- one-time prep -------------------------------------------------
    # x loads are casting fp32->bf16 SWDGE DMAs (gpsimd): queue the first
    # groups immediately -- they need no prep and pace the whole kernel.
    pre_x8 = []
    for gd in range(2):
        src0 = x_d[gd * 4 * P:(gd + 1) * 4 * P, :]
        src0 = src0.rearrange("(p t) d -> p t d", t=4)
        x80 = xbpool.tile([P, DMA_GROUP, D], BF16, tag="xb")
        nc.gpsimd.dma_start(x80[:, :4, :], src0)
        pre_x8.append(x80)

    ident_bf = const.tile([P, P], BF16)
    make_identity(nc, ident_bf[:])
    ident_f32 = const.tile([P, P], F32)
    make_identity(nc, ident_f32[:])

    # warm the ACT Square table set while DMAs are in flight (the
    # ACT_TABLE_LOAD costs ~2.7us and would otherwise sit in the prep
    # critical path at first use)
    warm = const.tile([1, 1], F32)
    nc.scalar.activation(warm[:], ident_f32[0:1, 0:1], AF.Square)

    c_sb = const.tile([C, D], F32)
    nc.sync.dma_start(c_sb[:], c_d[:])

    # c2 = rowsum(c^2) as a [128, 1] fp32 column
    c_sq = const.tile([C, D], F32)
    c2col = const.tile([C, 1], F32)
    nc.scalar.activation(c_sq[:], c_sb[:], AF.Square, accum_out=c2col[:])

    # (-2c) in bf16, then its transpose cT [d-chunk partitions, k, centers]
    cm2 = const.tile([C, D], BF16)
    nc.vector.tensor_scalar_mul(cm2[:], c_sb[:], -2.0)
    ct_ps = scr_ps.tile([P, 2, C], BF16, tag="scratch")
    for k in range(2):
        nc.tensor.transpose(ct_ps[:, k, :], cm2[:, k * P:(k + 1) * P], ident_bf[:])
    cT = const.tile([P, 2, C], BF16)
    nc.vector.tensor_copy(cT[:], ct_ps[:])

    # c2 as two bf16 K-rows (hi + lo) so a K=2 ones-matmul adds fp32-accurate c2
    c2t_ps = scr_ps.tile([1, C], F32, tag="scratch")
    nc.tensor.transpose(c2t_ps[:], c2col[:], ident_f32[:])
    c2row_f = const.tile([1, C], F32)
    nc.vector.tensor_copy(c2row_f[:], c2t_ps[:])
    c2rows = const.tile([2, C], BF16)
    nc.vector.tensor_copy(c2rows[0:1, :], c2row_f[:])
    c2hi_f = const.tile([1, C], F32)
    nc.vector.tensor_copy(c2hi_f[:], c2rows[0:1, :])
    c2lo_f = const.tile([1, C], F32)
    nc.vector.tensor_tensor(c2lo_f[:], c2row_f[:], c2hi_f[:], op=ALU.subtract)
    # engines can't write at base partition 1; a casting SBUF->SBUF DMA can
    nc.gpsimd.dma_start(c2rows[1:2, :], c2lo_f[:])

    ones2 = const.tile([2, C], BF16)
    nc.vector.memset(ones2[:], 1.0)
    ones_col = const.tile([P, 1], F32)
    nc.vector.memset(ones_col[:], 1.0)

    # c2rows replicated PSUM_GROUP times for the single N=512 c2 matmul
    c2rows4 = const.tile([2, PSUM_GROUP, C], BF16)
    for i in range(PSUM_GROUP):
        nc.vector.tensor_copy(c2rows4[:, i, :], c2rows[:])

    # all-ones [d, c] rhs for the PE-side x2 rank-1 accumulation
    ones_dc = const.tile([P, C], BF16)
    nc.vector.memset(ones_dc[:], 1.0)

    # semi_target: the HOST pre-permutes st into the x row mapping
    # (st_pre[p*NT + col] = st[row(p, col)], see make_in_maps), so one
    # contiguous 512B-per-partition DMA loads it.  The old direct load
    # of the scattered layout was 2048 32-byte descriptors (~14us) and
    # stalled the DVE queue behind the endgame's st-dependent ops.
    st_sb = const.tile([P, NT], F32)
    nc.sync.dma_start(st_sb[:], st_d[:].rearrange("(p j) -> p j", p=P))

    # per-b-tile accumulators: column j <-> b-tile j, partition p <-> row in tile
    mw = const.tile([P, NT], F32)
    x2w = const.tile([P, NT], F32)
    n_eg = len(EG_PLAN) + 1
    lsum2 = const.tile([P, n_eg], F32)
    # PE-x2 groups fold x2 into G before the min; their x2w columns
    # must read as zero in the endgame's dist = x2w + mw
    nc.vector.memset(x2w[:], 0.0)

    # ---- endgame (runs in chunks; all but the last overlap the main loop)
    def endgame_chunk(h, lo, hi):
        cols = slice(lo, hi)
        W = hi - lo
        dist = endp.tile([P, W], F32, tag=f"dist{h}")
        nc.vector.tensor_tensor(dist[:], x2w[:, cols], mw[:, cols], op=ALU.add)
        nc.vector.tensor_scalar_max(dist[:], dist[:], 0.0)
        dp = endp.tile([P, W], F32, tag=f"dp{h}")
        nc.vector.tensor_scalar_add(dp[:], dist[:], EPS)
        r = endp.tile([P, W], F32, tag=f"r{h}")
        nc.vector.reciprocal(r[:], dp[:])
        # loss = dist + min(st,0)*(dist - r) + max(st,0)*EPS
        t1 = endp.tile([P, W], F32, tag=f"t1{h}")
        nc.vector.tensor_tensor(t1[:], dist[:], r[:], op=ALU.subtract)
        mneg = endp.tile([P, W], F32, tag=f"mneg{h}")
        nc.vector.tensor_scalar_min(mneg[:], st_sb[:, cols], 0.0)
        t2 = endp.tile([P, W], F32, tag=f"t2{h}")
        nc.vector.tensor_tensor(t2[:], mneg[:], t1[:], op=ALU.mult)
        t3 = endp.tile([P, W], F32, tag=f"t3{h}")
        nc.vector.tensor_tensor(t3[:], dist[:], t2[:], op=ALU.add)
        epsq = endp.tile([P, W], F32, tag=f"eq{h}")
        nc.vector.tensor_scalar(epsq[:], st_sb[:, cols], 0.0, EPS, op0=ALU.max,
                                op1=ALU.mult)
        losses = endp.tile([P, W], F32, tag=f"lo{h}")
        nc.vector.tensor_tensor(losses[:], t3[:], epsq[:], op=ALU.add)
        nc.vector.tensor_reduce(lsum2[:, h:h + 1], losses[:],
                                axis=mybir.AxisListType.X, op=ALU.add)

    # ---- main loop -----------------------------------------------------
    # Edge groups are 4 tiles (0.5 MiB loads) so the pipeline ramps in and
    # drains out faster; the steady-state middle uses 8-tile groups.
    if repeat == 1 and hw_loop == 1:
        plan = [(t, 4) for t in range(0, 16, 4)]
        plan += [(t, 8) for t in range(16, 112, 8)]
        plan += [(t, 4) for t in range(112, 128, 4)]
    else:
        plan = [(t, 8) for t in range(0, NT, 8)]

    mini_idx = 0
    with tc.For_i(0, hw_loop, 1) if hw_loop > 1 else nullcontext():
     for _rep in range(repeat):
      for pi, (t0, ntile) in enumerate(plan):
        src = x_d[t0 * P:(t0 + ntile) * P, :]
        # row (p, t) of this group = batch t0*128 + p*ntile + t: each
        # partition reads one contiguous run per DMA
        src = src.rearrange("(p t) d -> p t d", t=ntile)
        if repeat == 1 and hw_loop == 1 and pi < len(pre_x8):
            x8 = pre_x8[pi]
        else:
            x8 = xbpool.tile([P, DMA_GROUP, D], BF16, tag="xb")
            nc.gpsimd.dma_start(x8[:, :ntile, :], src)

        cols = slice(t0, t0 + ntile)
        x2_on_pe = (pi % 3 != X2_SKIP)
        if not x2_on_pe:
            # sqred path: Square(x8) on ACT, then half-fold (bf16 2x TT)
            # + strided reduce-add on DVE
            sq = sqpool.tile([P, DMA_GROUP, D], BF16, tag="sq")
            nc.scalar.activation(sq[:, :ntile, :], x8[:, :ntile, :], AF.Square)
            s1 = sqpool.tile([P, DMA_GROUP, P], BF16, tag="s1")
            nc.vector.tensor_tensor(
                s1[:, :ntile, :], sq[:, :ntile, 0:P], sq[:, :ntile, P:D],
                op=ALU.add,
            )
            nc.vector.tensor_reduce(
                x2w[:, cols], s1[:, :ntile, :], axis=mybir.AxisListType.X,
                op=ALU.add,
            )

        g_ps = gps.tile([P, G_TILES, C], F32)
        for gp in range(ntile // PSUM_GROUP):
            tiles = [gp * PSUM_GROUP + t for t in range(PSUM_GROUP)]

            xt_ps = xtps.tile([P, PSUM_GROUP, 2, P], BF16)
            for i, t in enumerate(tiles):
                for k in range(2):
                    nc.tensor.transpose(
                        xt_ps[:, i, k, :], x8[:, t, k * P:(k + 1) * P],
                        ident_bf[:],
                    )
            xt_t = xtsb.tile([P, PSUM_GROUP, 2, P], BF16)
            # bf16 stays bf16 in PSUM; move it as fp32 pairs (half the
            # elements; exact on normals)
            cp_src = xt_ps[:].bitcast(F32)
            cp_dst = xt_t[:].bitcast(F32)
            if (mini_idx % 4) in COPY_DVE_MINIS:
                nc.vector.tensor_copy(cp_dst, cp_src)
            else:
                nc.scalar.copy(cp_dst, cp_src)
            mini_idx += 1

            if x2_on_pe:
                sqt = sqpool.tile([P, PSUM_GROUP, 2, P], BF16, tag="sqt")
                if pi % 3 == SQT_ACT_MOD:
                    nc.scalar.activation(sqt[:], xt_t[:], AF.Square)
                else:
                    nc.vector.tensor_tensor(sqt[:], xt_t[:], xt_t[:],
                                            op=ALU.mult)

            g_half = g_ps[:, gp * PSUM_GROUP:(gp + 1) * PSUM_GROUP, :]
            nc.tensor.matmul(
                g_half.rearrange("p t c -> p (t c)"),
                lhsT=ones2[:], rhs=c2rows4[:].rearrange("p t c -> p (t c)"),
                start=True, stop=False,
            )
            for i in range(PSUM_GROUP):
                last_tile = i == PSUM_GROUP - 1
                nc.tensor.matmul(
                    g_half[:, i, :], lhsT=xt_t[:, i, 0, :], rhs=cT[:, 0, :],
                    start=False, stop=False,
                )
                nc.tensor.matmul(
                    g_half[:, i, :], lhsT=xt_t[:, i, 1, :], rhs=cT[:, 1, :],
                    start=False, stop=(last_tile and not x2_on_pe),
                )
                if x2_on_pe:
                    nc.tensor.matmul(
                        g_half[:, i, :], lhsT=sqt[:, i, 0, :], rhs=ones_dc[:],
                        start=False, stop=False,
                    )
                    nc.tensor.matmul(
                        g_half[:, i, :], lhsT=sqt[:, i, 1, :], rhs=ones_dc[:],
                        start=False, stop=last_tile,
                    )

        nc.vector.tensor_reduce(
            mw[:, cols], g_ps[:, :ntile, :], axis=mybir.AxisListType.X,
            op=ALU.min,
        )

        if repeat == 1 and hw_loop == 1:
            for h, (trig, lo, hi) in enumerate(EG_PLAN):
                if t0 + ntile == trig:
                    endgame_chunk(h, lo, hi)

    endgame_chunk(len(EG_PLAN), *EG_FINAL)
    lacc = lsum2[:, 0:1]
    lsum_t = None
    for h in range(1, n_eg):
        nxt = endp.tile([P, 1], F32, tag=f"ls{h}")
        nc.vector.tensor_tensor(nxt[:], lacc, lsum2[:, h:h + 1], op=ALU.add)
        lacc = nxt[:]
        lsum_t = nxt
    # single-descriptor 4-byte out DMA: a [128,1] out would be 128 tiny
    # descriptors whose completion receipt stalls the end barrier ~7us
    total_ps = scr_ps.tile([1, 1], F32, tag="scratch")
    nc.tensor.matmul(total_ps[:], lhsT=ones_col[:], rhs=lsum_t[:])
    total_sb = endp.tile([1, 1], F32)
    nc.vector.tensor_copy(total_sb[:], total_ps[:])
    nc.sync.dma_start(out_d[:], total_sb[:])


def build_nc(repeat: int = 1, hw_loop: int = 1, internal_x: bool = False):
    key = (repeat, hw_loop, internal_x)
    if key in _cached_nc:
        return _cached_nc[key]
    nc = bacc.Bacc(
        "TRN2",
        target_bir_lowering=False,
        debug=False,
        enable_asserts=False,
        num_devices=N_CORES,
    )
    if internal_x:
        # timing-only builds: x is internal (uninitialized) DRAM so bench
        # calls don't upload 128 MiB; compute timing is data-independent
        x_d = nc.dram_tensor("x", [B_SH, D], F32).ap()
    else:
        x_d = nc.dram_tensor("x", [B_SH, D], F32, kind="ExternalInput").ap()
    c_d = nc.dram_tensor("c", [C, D], F32, kind="ExternalInput").ap()
    st_d = nc.dram_tensor("st", [B_SH], F32, kind="ExternalInput").ap()
    out_d = nc.dram_tensor("out", [1, 1], F32, kind="ExternalOutput").ap()

    with tile.TileContext(nc) as tc:
        with ExitStack() as ctx:
            _emit(ctx, tc, x_d, c_d, st_d, out_d, repeat=repeat, hw_loop=hw_loop)
    nc.compile()
    _cached_nc[key] = nc
    return nc


_ST_IDX = None


def _st_index():
    # row index feeding st_sb[p, col]: in an ntile-tile group at tile t0,
    # batch row t0*128 + p*ntile + t sits at column t0 + t
    global _ST_IDX
    if _ST_IDX is None:
        idx = np.empty((P, NT), dtype=np.int64)
        p = np.arange(P)[:, None]
        for lo, hi, tt in ((0, 16, 4), (16, 112, 8), (112, 128, 4)):
            for g0 in range(lo, hi, tt):
                t = np.arange(tt)[None, :]
                idx[:, g0:g0 + tt] = g0 * P + p * tt + t
        _ST_IDX = idx.ravel()
    return _ST_IDX


def make_in_maps(x, c, stf):
    idx = _st_index()
    return [
        {
            "x": np.ascontiguousarray(x[i * B_SH:(i + 1) * B_SH]),
            "c": c,
            "st": np.ascontiguousarray(stf[i * B_SH:(i + 1) * B_SH][idx]),
        }
        for i in range(N_CORES)
    ]


def kernel(**inputs) -> np.ndarray:
    x = np.ascontiguousarray(np.asarray(inputs["input"], dtype=np.float32))
    c = np.ascontiguousarray(np.asarray(inputs["c"], dtype=np.float32))
    stf = np.asarray(inputs["semi_target"]).astype(np.float32)

    nc = build_nc()
    res = run_bass_kernel_spmd(nc, make_in_maps(x, c, stf), list(range(N_CORES)))
    total = sum(float(r["out"][0, 0]) for r in res.results)
    return np.asarray(np.float32(total / B))


# revision 18
# speedup vs baseline: 1.1284x; 1.0827x over previous
"""DMSAD loss kernel for Trainium2 (8 NeuronCores, data-parallel over batch).

Computes mean over B rows of:
    dist_i = max(min_j ||x_i - c_j||^2, 0)
    loss_i = dist_i                 if st_i == 0
             dist_i + EPS           if st_i == 1
             1 / (dist_i + EPS)     if st_i == -1

Per core (B_SH = 16384 rows, D = 256, C = 128), engine pipeline:
  - DMA (SWDGE/gpsimd): casting fp32->bf16 HBM loads -- the 16.8 MB fp32
    read paces at the ~47us HBM roofline while landing bf16 directly in
    SBUF, deleting the ACT/DVE cast stage entirely.
  - PE: transposes are issued as NORMAL matmuls against an identity rhs
    (out = lhsT.T @ I) rather than transpose-mode ops -- transpose-mode
    does not count as PE-busy for the HAM clock gate, and with it the
    PE kept falling back to the cold 1.2 GHz clock mid-stream.  Normal
    matmuls keep HAM at 8/8 (2.4 GHz).  Output lands fp32 in PSUM
    (2-tile minigroups, 1 bank each); main bf16 matmuls G += -2 x.c^T;
    a K=2 ones x [c2_hi; c2_lo] matmul folds the center norms in; for
    most groups a rank-1 ones-matmul of the squared transposed tiles
    folds x2 into G so dist falls out of the min directly.
  - ACT: casting fp32->bf16 PSUM->SBUF copies of the transposed tiles,
    Square for part of the sqt work and the sqred groups.
  - DVE: part of the copies/sqt, batched min-reduce per 8-tile G tile,
    half-fold + reduce-add x2 for sqred groups, endgame in small chunks
    overlapped with the main loop.
A final ones-matmul collapses the per-partition loss sums to one scalar
per core (single-descriptor 4-byte out DMA); host adds the 8 partials.
"""

from contextlib import ExitStack, nullcontext

import numpy as np

import concourse.bass as bass
import concourse.tile as tile
from concourse import bacc, mybir
from concourse.bass_utils import run_bass_kernel_spmd
from concourse.masks import make_identity

N_CORES = 8
B = 131072
D = 256
C = 128
P = 128
B_SH = B // N_CORES          # 16384 rows per core
NT = B_SH // P               # 128 b-tiles of 128 rows
MINI = 2                     # b-tiles per transpose PSUM minigroup (1 bank fp32)
PSUM_GROUP = 4               # b-tiles per G PSUM bank
DMA_GROUP = 8                # b-tiles per input DMA (1 MiB fp32 reads)
G_TILES = 8                  # b-tiles per G PSUM tile (2 banks, one min-reduce)
ETA = 1.0
EPS = 1e-6

# ---- engine-balance knobs -------------------------------------------------
# x2-on-PE pattern: group pi uses the PE rank-1 path unless pi % 4 == X2_SKIP
# (sqred groups compute x2 fully on DVE: TT square + half-fold + reduce --
#  keeping the ACT FIFO free of big ops so psum-copies never stall the PE)
X2_SKIP = 2
X2_MOD = 4
# sqt (squares of transposed tiles): of every 4 psum-groups, this many on ACT
SQT_ACT_OF_4 = 2
# PSUM->SBUF pair copies: psum-group indices (mod 4) that go to DVE
COPY_DVE_MINIS = ()
# endgame trigger points (t0+ntile values) and column chunks
EG_PLAN = [(32, 0, 32), (64, 32, 64), (96, 64, 96), (120, 96, 120)]
EG_FINAL = (120, 128)

F32 = mybir.dt.float32
BF16 = mybir.dt.bfloat16
AF = mybir.ActivationFunctionType
ALU = mybir.AluOpType

_cached_nc = {}


def _emit(ctx: ExitStack, tc, x_d, c_d, st_d, out_d, repeat: int = 1,
          hw_loop: int = 1):
    nc = tc.nc

    const = ctx.enter_context(tc.tile_pool(name="const", bufs=1))
    xbpool = ctx.enter_context(tc.tile_pool(name="xb", bufs=6))
    sqpool = ctx.enter_context(tc.tile_pool(name="sq", bufs=3))
    xtps = ctx.enter_context(tc.tile_pool(name="xtps", bufs=3, space="PSUM"))
    xtsb = ctx.enter_context(tc.tile_pool(name="xtsb", bufs=3))
    # G lives in 2-bank [P, 8, C] tiles so the min-reduce batches a whole
    # DMA group.
    gps = ctx.enter_context(tc.tile_pool(name="gps", bufs=2, space="PSUM"))
    scr_ps = ctx.enter_context(tc.tile_pool(name="scrps", bufs=1, space="PSUM"))
    endp = ctx.enter_context(tc.tile_pool(name="endp", bufs=1))

    # ---- one-time prep -------------------------------------------------
    # x loads are casting fp32->bf16 SWDGE DMAs (gpsimd): queue the first
    # groups immediately -- they need no prep and pace the whole kernel.
    pre_x8 = []
    for gd in range(2):
        src0 = x_d[gd * 4 * P:(gd + 1) * 4 * P, :]
        src0 = src0.rearrange("(p t) d -> p t d", t=4)
        x80 = xbpool.tile([P, DMA_GROUP, D], BF16, tag="xb")
        nc.gpsimd.dma_start(x80[:, :4, :], src0)
        pre_x8.append(x80)

    ident_bf = const.tile([P, P], BF16)
    make_identity(nc, ident_bf[:])
    ident_f32 = const.tile([P, P], F32)
    make_identity(nc, ident_f32[:])

    # warm the ACT Square table set while DMAs are in flight (the
    # ACT_TABLE_LOAD costs ~2.7us and would otherwise sit in the prep
    # critical path at first use)
    warm = const.tile([1, 1], F32)
    nc.scalar.activation(warm[:], ident_f32[0:1, 0:1], AF.Square)

    c_sb = const.tile([C, D], F32)
    nc.sync.dma_start(c_sb[:], c_d[:])

    # c2 = rowsum(c^2) as a [128, 1] fp32 column
    c_sq = const.tile([C, D], F32)
    c2col = const.tile([C, 1], F32)
    nc.scalar.activation(c_sq[:], c_sb[:], AF.Square, accum_out=c2col[:])

    # (-2c) in bf16, then its transpose cT [d-chunk partitions, k, centers]
    cm2 = const.tile([C, D], BF16)
    nc.vector.tensor_scalar_mul(cm2[:], c_sb[:], -2.0)
    ct_ps = scr_ps.tile([P, 2, C], BF16, tag="scratch")
    for k in range(2):
        nc.tensor.transpose(ct_ps[:, k, :], cm2[:, k * P:(k + 1) * P], ident_bf[:])
    cT = const.tile([P, 2, C], BF16)
    nc.vector.tensor_copy(cT[:], ct_ps[:])

    # c2 as two bf16 K-rows (hi + lo) so a K=2 ones-matmul adds fp32-accurate c2
    c2t_ps = scr_ps.tile([1, C], F32, tag="scratch")
    nc.tensor.transpose(c2t_ps[:], c2col[:], ident_f32[:])
    c2row_f = const.tile([1, C], F32)
    nc.vector.tensor_copy(c2row_f[:], c2t_ps[:])
    c2rows = const.tile([2, C], BF16)
    nc.vector.tensor_copy(c2rows[0:1, :], c2row_f[:])
    c2hi_f = const.tile([1, C], F32)
    nc.vector.tensor_copy(c2hi_f[:], c2rows[0:1, :])
    c2lo_f = const.tile([1, C], F32)
    nc.vector.tensor_tensor(c2lo_f[:], c2row_f[:], c2hi_f[:], op=ALU.subtract)
    # engines can't write at base partition 1; a casting SBUF->SBUF DMA can
    nc.gpsimd.dma_start(c2rows[1:2, :], c2lo_f[:])

    ones2 = const.tile([2, C], BF16)
    nc.vector.memset(ones2[:], 1.0)
    ones_col = const.tile([P, 1], F32)
    nc.vector.memset(ones_col[:], 1.0)

    # c2rows replicated PSUM_GROUP times for the single N=512 c2 matmul
    c2rows4 = const.tile([2, PSUM_GROUP, C], BF16)
    nc.vector.tensor_copy(c2rows4[:, 0, :], c2rows[:])
    nc.vector.tensor_copy(c2rows4[:, 1, :], c2rows[:])
    nc.vector.tensor_copy(c2rows4[:, 2:4, :], c2rows4[:, 0:2, :])

    # all-ones [d, c] rhs for the PE-side x2 rank-1 accumulation
    ones_dc = const.tile([P, C], BF16)
    nc.vector.memset(ones_dc[:], 1.0)

    # semi_target: the HOST pre-permutes st into the x row mapping
    # (st_pre[p*NT + col] = st[row(p, col)], see make_in_maps), so one
    # contiguous 512B-per-partition DMA loads it.  The old direct load
    # of the scattered layout was 2048 32-byte descriptors (~14us) and
    # stalled the DVE queue behind the endgame's st-dependent ops.
    st_sb = const.tile([P, NT], F32)
    nc.sync.dma_start(st_sb[:], st_d[:].rearrange("(p j) -> p j", p=P))

    # per-b-tile accumulators: column j <-> b-tile j, partition p <-> row in tile
    mw = const.tile([P, NT], F32)
    x2w = const.tile([P, NT], F32)
    n_eg = len(EG_PLAN) + 1
    lsum2 = const.tile([P, n_eg], F32)
    # PE-x2 groups fold x2 into G before the min; their x2w columns
    # must read as zero in the endgame's dist = x2w + mw
    nc.vector.memset(x2w[:], 0.0)

    # ---- endgame (runs in chunks; all but the last overlap the main loop)
    def endgame_chunk(h, lo, hi):
        cols = slice(lo, hi)
        W = hi - lo
        dist = endp.tile([P, W], F32, tag=f"dist{h}")
        nc.vector.tensor_tensor(dist[:], x2w[:, cols], mw[:, cols], op=ALU.add)
        nc.vector.tensor_scalar_max(dist[:], dist[:], 0.0)
        dp = endp.tile([P, W], F32, tag=f"dp{h}")
        nc.vector.tensor_scalar_add(dp[:], dist[:], EPS)
        r = endp.tile([P, W], F32, tag=f"r{h}")
        nc.vector.reciprocal(r[:], dp[:])
        # loss = dist + min(st,0)*(dist - r) + max(st,0)*EPS
        t1 = endp.tile([P, W], F32, tag=f"t1{h}")
        nc.vector.tensor_tensor(t1[:], dist[:], r[:], op=ALU.subtract)
        mneg = endp.tile([P, W], F32, tag=f"mneg{h}")
        nc.vector.tensor_scalar_min(mneg[:], st_sb[:, cols], 0.0)
        t2 = endp.tile([P, W], F32, tag=f"t2{h}")
        nc.vector.tensor_tensor(t2[:], mneg[:], t1[:], op=ALU.mult)
        t3 = endp.tile([P, W], F32, tag=f"t3{h}")
        nc.vector.tensor_tensor(t3[:], dist[:], t2[:], op=ALU.add)
        epsq = endp.tile([P, W], F32, tag=f"eq{h}")
        nc.vector.tensor_scalar(epsq[:], st_sb[:, cols], 0.0, EPS, op0=ALU.max,
                                op1=ALU.mult)
        losses = endp.tile([P, W], F32, tag=f"lo{h}")
        nc.vector.tensor_tensor(losses[:], t3[:], epsq[:], op=ALU.add)
        nc.vector.tensor_reduce(lsum2[:, h:h + 1], losses[:],
                                axis=mybir.AxisListType.X, op=ALU.add)

    # ---- main loop -----------------------------------------------------
    # Edge groups are 4 tiles (0.5 MiB loads) so the pipeline ramps in and
    # drains out faster; the steady-state middle uses 8-tile groups.
    if repeat == 1 and hw_loop == 1:
        plan = [(t, 4) for t in range(0, 16, 4)]
        plan += [(t, 8) for t in range(16, 112, 8)]
        plan += [(t, 4) for t in range(112, 128, 4)]
    else:
        plan = [(t, 8) for t in range(0, NT, 8)]

    mini_idx = 0
    with tc.For_i(0, hw_loop, 1) if hw_loop > 1 else nullcontext():
     for _rep in range(repeat):
      for pi, (t0, ntile) in enumerate(plan):
        src = x_d[t0 * P:(t0 + ntile) * P, :]
        # row (p, t) of this group = batch t0*128 + p*ntile + t: each
        # partition reads one contiguous run per DMA
        src = src.rearrange("(p t) d -> p t d", t=ntile)
        if repeat == 1 and hw_loop == 1 and pi < len(pre_x8):
            x8 = pre_x8[pi]
        else:
            x8 = xbpool.tile([P, DMA_GROUP, D], BF16, tag="xb")
            nc.gpsimd.dma_start(x8[:, :ntile, :], src)

        cols = slice(t0, t0 + ntile)
        x2_on_pe = (pi % X2_MOD != X2_SKIP)
        if not x2_on_pe:
            # sqred path, all on DVE (big ACT ops would stall the psum
            # copies in the ACT FIFO and starve the PE): TT square (bf16
            # 2x), half-fold, strided reduce-add
            sq = sqpool.tile([P, DMA_GROUP, D], BF16, tag="sq")
            nc.vector.tensor_tensor(sq[:, :ntile, :], x8[:, :ntile, :],
                                    x8[:, :ntile, :], op=ALU.mult)
            s1 = sqpool.tile([P, DMA_GROUP, P], BF16, tag="s1")
            nc.vector.tensor_tensor(
                s1[:, :ntile, :], sq[:, :ntile, 0:P], sq[:, :ntile, P:D],
                op=ALU.add,
            )
            nc.vector.tensor_reduce(
                x2w[:, cols], s1[:, :ntile, :], axis=mybir.AxisListType.X,
                op=ALU.add,
            )

        g_ps = gps.tile([P, G_TILES, C], F32)
        for gp in range(ntile // PSUM_GROUP):
            tiles = [gp * PSUM_GROUP + t for t in range(PSUM_GROUP)]

            xt_ps = xtps.tile([P, PSUM_GROUP, 2, P], BF16)
            for i, t in enumerate(tiles):
                for k in range(2):
                    nc.tensor.transpose(
                        xt_ps[:, i, k, :], x8[:, t, k * P:(k + 1) * P],
                        ident_bf[:],
                    )
            xt_t = xtsb.tile([P, PSUM_GROUP, 2, P], BF16)
            # bf16 stays bf16 in PSUM; move it as fp32 pairs (half the
            # elements; exact on normals)
            cp_src = xt_ps[:].bitcast(F32)
            cp_dst = xt_t[:].bitcast(F32)
            if (mini_idx % 4) in COPY_DVE_MINIS:
                nc.vector.tensor_copy(cp_dst, cp_src)
            else:
                nc.scalar.copy(cp_dst, cp_src)

            if x2_on_pe:
                sqt = sqpool.tile([P, PSUM_GROUP, 2, P], BF16, tag="sqt")
                if (mini_idx % 4) < SQT_ACT_OF_4:
                    nc.scalar.activation(sqt[:], xt_t[:], AF.Square)
                else:
                    nc.vector.tensor_tensor(sqt[:], xt_t[:], xt_t[:],
                                            op=ALU.mult)
            mini_idx += 1

            g_half = g_ps[:, gp * PSUM_GROUP:(gp + 1) * PSUM_GROUP, :]
            nc.tensor.matmul(
                g_half.rearrange("p t c -> p (t c)"),
                lhsT=ones2[:], rhs=c2rows4[:].rearrange("p t c -> p (t c)"),
                start=True, stop=False,
            )
            for i in range(PSUM_GROUP):
                last_tile = i == PSUM_GROUP - 1
                nc.tensor.matmul(
                    g_half[:, i, :], lhsT=xt_t[:, i, 0, :], rhs=cT[:, 0, :],
                    start=False, stop=False,
                )
                nc.tensor.matmul(
                    g_half[:, i, :], lhsT=xt_t[:, i, 1, :], rhs=cT[:, 1, :],
                    start=False, stop=(last_tile and not x2_on_pe),
                )
                if x2_on_pe:
                    nc.tensor.matmul(
                        g_half[:, i, :], lhsT=sqt[:, i, 0, :], rhs=ones_dc[:],
                        start=False, stop=False,
                    )
                    nc.tensor.matmul(
                        g_half[:, i, :], lhsT=sqt[:, i, 1, :], rhs=ones_dc[:],
                        start=False, stop=last_tile,
                    )

        nc.vector.tensor_reduce(
            mw[:, cols], g_ps[:, :ntile, :], axis=mybir.AxisListType.X,
            op=ALU.min,
        )

        if repeat == 1 and hw_loop == 1:
            for h, (trig, lo, hi) in enumerate(EG_PLAN):
                if t0 + ntile == trig:
                    endgame_chunk(h, lo, hi)

    endgame_chunk(len(EG_PLAN), *EG_FINAL)
    lacc = lsum2[:, 0:1]
    lsum_t = None
    for h in range(1, n_eg):
        nxt = endp.tile([P, 1], F32, tag=f"ls{h}")
        nc.vector.tensor_tensor(nxt[:], lacc, lsum2[:, h:h + 1], op=ALU.add)
        lacc = nxt[:]
        lsum_t = nxt
    # single-descriptor 4-byte out DMA: a [128,1] out would be 128 tiny
    # descriptors whose completion receipt stalls the end barrier ~7us
    total_ps = scr_ps.tile([1, 1], F32, tag="scratch")
    nc.tensor.matmul(total_ps[:], lhsT=ones_col[:], rhs=lsum_t[:])
    total_sb = endp.tile([1, 1], F32)
    nc.vector.tensor_copy(total_sb[:], total_ps[:])
    nc.sync.dma_start(out_d[:], total_sb[:])


def build_nc(repeat: int = 1, hw_loop: int = 1, internal_x: bool = False):
    key = (repeat, hw_loop, internal_x)
    if key in _cached_nc:
        return _cached_nc[key]
    nc = bacc.Bacc(
        "TRN2",
        target_bir_lowering=False,
        debug=False,
        enable_asserts=False,
        num_devices=N_CORES,
    )
    if internal_x:
        # timing-only builds: x is internal (uninitialized) DRAM so bench
        # calls don't upload 128 MiB; compute timing is data-independent
        x_d = nc.dram_tensor("x", [B_SH, D], F32).ap()
    else:
        x_d = nc.dram_tensor("x", [B_SH, D], F32, kind="ExternalInput").ap()
    c_d = nc.dram_tensor("c", [C, D], F32, kind="ExternalInput").ap()
    st_d = nc.dram_tensor("st", [B_SH], F32, kind="ExternalInput").ap()
    out_d = nc.dram_tensor("out", [1, 1], F32, kind="ExternalOutput").ap()

    with tile.TileContext(nc) as tc:
        with ExitStack() as ctx:
            _emit(ctx, tc, x_d, c_d, st_d, out_d, repeat=repeat, hw_loop=hw_loop)
    nc.compile()
    _cached_nc[key] = nc
    return nc


_ST_IDX = None


def _st_index():
    # row index feeding st_sb[p, col]: in an ntile-tile group at tile t0,
    # batch row t0*128 + p*ntile + t sits at column t0 + t
    global _ST_IDX
    if _ST_IDX is None:
        idx = np.empty((P, NT), dtype=np.int64)
        p = np.arange(P)[:, None]
        for lo, hi, tt in ((0, 16, 4), (16, 112, 8), (112, 128, 4)):
            for g0 in range(lo, hi, tt):
                t = np.arange(tt)[None, :]
                idx[:, g0:g0 + tt] = g0 * P + p * tt + t
        _ST_IDX = idx.ravel()
    return _ST_IDX


def make_in_maps(x, c, stf):
    idx = _st_index()
    return [
        {
            "x": np.ascontiguousarray(x[i * B_SH:(i + 1) * B_SH]),
            "c": c,
            "st": np.ascontiguousarray(stf[i * B_SH:(i + 1) * B_SH][idx]),
        }
        for i in range(N_CORES)
    ]


def kernel(**inputs) -> np.ndarray:
    x = np.ascontiguousarray(np.asarray(inputs["input"], dtype=np.float32))
    c = np.ascontiguousarray(np.asarray(inputs["c"], dtype=np.float32))
    stf = np.asarray(inputs["semi_target"]).astype(np.float32)

    nc = build_nc()
    res = run_bass_kernel_spmd(nc, make_in_maps(x, c, stf), list(range(N_CORES)))
    total = sum(float(r["out"][0, 0]) for r in res.results)
    return np.asarray(np.float32(total / B))
